# revision 11
# baseline (speedup 1.0000x reference)
"""DeepSet-equivariant layer on 8 TRN2 NeuronCores.

Math (reference):
    y = x @ w1 + (colsum(x) @ w2) / n + bias        x: (n, 128)

Distribution strategy (sharding_hint): shard x and y along the set
dimension n across the 8 cores; each core computes its local column-sum,
an AllGather + local reduce produces the global pooled vector, and
w1/w2/bias are replicated.

Device layout trick: each core receives its shard PRE-TRANSPOSED on the
host as xT (128=d_in partitions, rows free) and returns yT in the same
layout.  This makes the kernel transpose-free:
  - main matmul: lhsT = w1 (stationary), rhs = xT chunk -> yT in PSUM
  - column-sum  = free-dim reduce of xT
  - transmit+bias is a per-partition column in yT layout.

The x / w1 / w2 / y streams are bf16 (host casts both ways; rel err
~3e-3, well inside the 2e-2 gate) which HALVES the HBM traffic per
core.  All accumulation stays fp32 (PSUM / reduce accumulators).

Schedule: the collective latency (ncfw wake + cross-core launch-skew
barrier + AllGather, 40-120 us) dominates, so EVERYTHING that does not
depend on the global pooled vector runs before/during the wait:
  - a warm-up AllGather issued before the TileContext absorbs the ncfw
    wake + barrier from t~7us;
  - phase 1 streams x into SBUF (DMA-bound ~20 us) with column-sum
    slices on DVE/ACT; the 512 B local sum AllGathers at ~30 us;
  - the full matmul x@w1 runs during the wait, PSUM draining via pure
    copies into a resident bf16 staging buffer y0 (software-pipelined
    512-col matmuls / copies over one 8-bank PSUM tile).
After the AllGather lands, the gathered [8,128] block loads as one
clean 8-partition DMA, the cross-core sum is a single K=8 matmul with
a ones vector, and the only per-element work left is y0 + t -> out:
bf16 SBUF->SBUF tensor_scalar (DVE 2x/4x packing eligible) alternating
with ACT, feeding ramped output DMA chunks.  The post-collective tail
is then output-DMA-bound (~22 us) instead of Tensor-bound.
"""

import numpy as np
import ml_dtypes

import concourse.bass as bass
import concourse.tile as tile
from concourse import bacc, mybir
from concourse.bass_utils import run_bass_kernel_spmd

N_CORES = 8
D = 128                 # d_in == d_out
N_ROWS = 200000         # full set size
R = 25088               # padded rows per core: 8 * 25088 = 200704 >= 200000
IN_CHUNK = 8192         # columns per input DMA chunk (2 MiB bf16)
RED_CHUNK = 2048        # columns per column-sum reduce slice
MM_N = 512              # matmul free dim = copy-drain width (one PSUM bank)
PS_COLS = 4096          # one resident PSUM tile, all 8 banks
NPIPE = PS_COLS // MM_N  # matmuls run this many 512-col slices ahead of copies
OUT_CHUNKS = [2048, 4096, 8192, 8192, 2560]   # ramped output DMA chunks
assert sum(OUT_CHUNKS) == R

F32 = mybir.dt.float32
BF16 = mybir.dt.bfloat16
NP_BF16 = ml_dtypes.bfloat16


def _split(r, step, base=0):
    out = []
    c0 = 0
    while c0 < r:
        cw = min(step, r - c0)
        out.append((base + c0, cw))
        c0 += cw
    return out


def build_nc(r: int, n_total: int):
    """Build the SPMD Bass program for one core holding r rows."""
    in_chunks = _split(r, IN_CHUNK)

    nc = bacc.Bacc(
        "TRN2",
        target_bir_lowering=False,
        debug=False,
        num_devices=N_CORES,
    )

    xt = nc.declare_dram_parameter("xt", [D, r], BF16, isOutput=False)
    w1 = nc.declare_dram_parameter("w1", [D, D], BF16, isOutput=False)
    w2 = nc.declare_dram_parameter("w2", [D, D], BF16, isOutput=False)
    bias_c = nc.declare_dram_parameter("bias_c", [D, 1], F32, isOutput=False)
    ones8 = nc.declare_dram_parameter("ones8", [N_CORES, 1], BF16, isOutput=False)
    out = nc.declare_dram_parameter("out", [D, r], BF16, isOutput=True)

    # Bounce buffers for the collectives (cannot touch I/O tensors).
    # AllGather output is rank-major: rank r's block at row r.
    cc_in = nc.dram_tensor("cc_in", [D, 1], BF16)
    cc_out = nc.dram_tensor("cc_out", [N_CORES, D], BF16, addr_space="Shared")
    ccw_in = nc.dram_tensor("ccw_in", [D, 1], F32)
    ccw_out = nc.dram_tensor("ccw_out", [N_CORES, D], F32, addr_space="Shared")

    # Warm-up collective, emitted BEFORE the TileContext so it is the very
    # first gpsimd instruction: wakes ncfw and enters the cross-core
    # barrier immediately at kernel start.  Content/result unused;
    # completion is guaranteed before the real AllGather because ncfw
    # serializes collectives in program order.
    warm_sem = nc.alloc_semaphore("warm_cc")
    nc.gpsimd.collective_compute(
        "AllGather",
        mybir.AluOpType.bypass,
        replica_groups=[list(range(N_CORES))],
        ins=[ccw_in.ap().opt()],
        outs=[ccw_out.ap().opt()],
    ).then_inc(warm_sem)

    with tile.TileContext(nc) as tc:
        with (
            tc.tile_pool(name="const", bufs=1) as const_pool,
            tc.tile_pool(name="xres", bufs=1) as xres_pool,
            tc.tile_pool(name="ystage", bufs=1) as y_pool,
            tc.tile_pool(name="obuf", bufs=3) as obuf_pool,
            tc.tile_pool(name="small", bufs=1) as small_pool,
            tc.tile_pool(name="mm", bufs=1, space=bass.MemorySpace.PSUM) as mm_pool,
        ):
            w1_sb = const_pool.tile([D, D], BF16)
            w2_sb = const_pool.tile([D, D], BF16)
            bias_sb = const_pool.tile([D, 1], F32)
            ones8_sb = const_pool.tile([N_CORES, 1], BF16)
            nc.scalar.dma_start(w1_sb[:], w1[:, :])
            nc.scalar.dma_start(w2_sb[:], w2[:, :])
            nc.scalar.dma_start(bias_sb[:], bias_c[:, :])
            nc.scalar.dma_start(ones8_sb[:], ones8[:, :])

            x_sb = xres_pool.tile([D, r], BF16)
            y0 = y_pool.tile([D, r], BF16)

            # phase 1: stream ALL of xT into SBUF (alternating HWDGE rings)
            for c, (c0, cw) in enumerate(in_chunks):
                dma_eng = nc.sync if c % 2 == 0 else nc.scalar
                dma_eng.dma_start(x_sb[:, c0 : c0 + cw], xt[:, c0 : c0 + cw])

            # column-sum slices, emitted BEFORE the copy-drains so they
            # lead each engine's FIFO; the last slice + combine go on ACT
            # so DVE's copy stream starts as early as possible.
            red_slices = _split(r, RED_CHUNK)
            n_red = len(red_slices)
            cs_parts = small_pool.tile([D, n_red], F32)
            trash = small_pool.tile([D, RED_CHUNK], BF16)
            for s, (s0, sw) in enumerate(red_slices):
                if s % 2 == 0 and s != n_red - 1:
                    nc.vector.reduce_sum(
                        cs_parts[:, s : s + 1],
                        x_sb[:, s0 : s0 + sw],
                        axis=mybir.AxisListType.X,
                    )
                else:
                    nc.scalar.activation(
                        trash[:, :sw],
                        x_sb[:, s0 : s0 + sw],
                        mybir.ActivationFunctionType.Copy,
                        accum_out=cs_parts[:, s : s + 1],
                    )
            cs = small_pool.tile([D, 1], F32)
            nc.scalar.activation(
                trash[:, :n_red],
                cs_parts[:],
                mybir.ActivationFunctionType.Copy,
                accum_out=cs[:],
            )
            cs_bf = small_pool.tile([D, 1], BF16)
            nc.scalar.activation(
                cs_bf[:],
                cs[:],
                mybir.ActivationFunctionType.Copy,
            )
            nc.sync.dma_start(cc_in[:, :], cs_bf[:])

            nc.gpsimd.collective_compute(
                "AllGather",
                mybir.AluOpType.bypass,
                replica_groups=[list(range(N_CORES))],
                ins=[cc_in.ap().opt()],
                outs=[cc_out.ap().opt()],
            )

            # gathered [8, D] block loads as one clean 8-partition DMA
            g8 = small_pool.tile([N_CORES, D], BF16)
            nc.sync.dma_start(g8[:, :], cc_out[:, :])

            # staged matmul: x@w1 -> PSUM -> pure-copy drain into y0, all
            # independent of the collective.  Software-pipelined emission:
            # copy(k) after mm(k); mm(k+NPIPE) — which overwrites group
            # k's PSUM bank — after copy(k).
            ps = mm_pool.tile([D, PS_COLS], F32, tag="ps")
            slices = _split(r, MM_N)

            def emit_mm(idx):
                s0, sw = slices[idx]
                po = s0 % PS_COLS
                nc.tensor.matmul(ps[:, po : po + sw], w1_sb[:], x_sb[:, s0 : s0 + sw])

            for k in range(min(NPIPE, len(slices))):
                emit_mm(k)
            for k, (s0, sw) in enumerate(slices):
                po = s0 % PS_COLS
                if k % 2 == 0:
                    nc.vector.tensor_copy(out=y0[:, s0 : s0 + sw], in_=ps[:, po : po + sw])
                else:
                    nc.scalar.activation(
                        y0[:, s0 : s0 + sw],
                        ps[:, po : po + sw],
                        mybir.ActivationFunctionType.Copy,
                    )
                if k + NPIPE < len(slices):
                    emit_mm(k + NPIPE)

            # cross-core sum: pool = ones8-weighted sum of the 8 gathered
            # rows (single K=8 matmul); then t = (w2.T @ pool/n) + bias,
            # produced in both bf16 (DVE adds) and fp32 (ACT bias path).
            nc.tensor.matmul(ps[:, 0:1], g8[:], ones8_sb[:])
            gcs = small_pool.tile([D, 1], BF16)
            nc.vector.tensor_scalar(
                out=gcs[:],
                in0=ps[:, 0:1],
                scalar1=1.0 / float(n_total),
                scalar2=None,
                op0=mybir.AluOpType.mult,
            )
            nc.tensor.matmul(ps[:, 1:2], w2_sb[:], gcs[:])
            t_f32 = small_pool.tile([D, 1], F32)
            nc.vector.tensor_scalar(
                out=t_f32[:],
                in0=ps[:, 1:2],
                scalar1=bias_sb[:],
                scalar2=None,
                op0=mybir.AluOpType.add,
            )

            # output: ob = y0 + t (bf16 SBUF->SBUF, alternating DVE/ACT),
            # then ramped DMA chunks on alternating rings.
            c0 = 0
            for c, cw in enumerate(OUT_CHUNKS):
                ob = obuf_pool.tile([D, max(OUT_CHUNKS)], BF16)
                if c != 2:
                    nc.vector.tensor_scalar(
                        out=ob[:, :cw],
                        in0=y0[:, c0 : c0 + cw],
                        scalar1=t_f32[:],
                        scalar2=None,
                        op0=mybir.AluOpType.add,
                    )
                else:
                    nc.scalar.activation(
                        ob[:, :cw],
                        y0[:, c0 : c0 + cw],
                        mybir.ActivationFunctionType.Identity,
                        bias=t_f32[:],
                        scale=1.0,
                    )
                (nc.sync if c % 2 == 0 else nc.scalar).dma_start(
                    out[:, c0 : c0 + cw], ob[:, :cw]
                )
                c0 += cw

    nc.compile()
    return nc


_nc_cache: dict = {}


def _get_nc(r: int, n_total: int):
    key = (r, n_total)
    if key not in _nc_cache:
        _nc_cache[key] = build_nc(r, n_total)
    return _nc_cache[key]


LAST_RESULTS = None


def _execute(x, w1, w2, bias, r, trace=False, tmpdir=None, trace_cores=None):
    global LAST_RESULTS
    x = np.ascontiguousarray(np.asarray(x, dtype=np.float32))
    w1 = np.asarray(w1, dtype=np.float32)
    w2 = np.asarray(w2, dtype=np.float32)
    bias = np.asarray(bias, dtype=np.float32)
    n, d = x.shape
    assert d == D and r * N_CORES >= n

    xp = np.zeros((N_CORES * r, d), dtype=np.float32)
    xp[:n] = x
    # (8, r, d) -> (8, d, r) pre-transposed bf16 shards
    xts = np.ascontiguousarray(
        xp.reshape(N_CORES, r, d).transpose(0, 2, 1).astype(NP_BF16)
    )
    w1_b = np.ascontiguousarray(w1.astype(NP_BF16))
    w2_b = np.ascontiguousarray(w2.astype(NP_BF16))
    bias_col = np.ascontiguousarray(bias.reshape(1, d).T)
    ones8_col = np.ones((N_CORES, 1), dtype=NP_BF16)

    in_maps = [
        {"xt": xts[i], "w1": w1_b, "w2": w2_b, "bias_c": bias_col, "ones8": ones8_col}
        for i in range(N_CORES)
    ]

    nc = _get_nc(r, n)
    kwargs = {}
    if trace:
        kwargs.update(trace=True, tmpdir=tmpdir)
        if trace_cores is not None:
            kwargs.update(trace_cores=trace_cores)
    res = run_bass_kernel_spmd(nc, in_maps, core_ids=list(range(N_CORES)), **kwargs)
    LAST_RESULTS = res

    yts = [res.results[i]["out"] for i in range(N_CORES)]  # each (D, r) bf16
    y = np.concatenate([yt.T.astype(np.float32) for yt in yts], axis=0)[:n]
    return np.ascontiguousarray(y)


def kernel(x, w1, w2, bias):
    return _execute(x, w1, w2, bias, R)


# revision 12
# speedup vs baseline: 1.1841x; 1.1841x over previous
"""DeepSet-equivariant layer on 8 TRN2 NeuronCores.

Math (reference):
    y = x @ w1 + (colsum(x) @ w2) / n + bias        x: (n, 128)

Distribution strategy (sharding_hint): shard x and y along the set
dimension n across the 8 cores; each core computes its local column-sum,
an AllGather + local reduce produces the global pooled vector, and
w1/w2/bias are replicated.

Device layout trick: each core receives its shard PRE-TRANSPOSED on the
host as xT (128=d_in partitions, rows free) and returns yT in the same
layout.  This makes the kernel transpose-free:
  - main matmul: lhsT = w1 (stationary), rhs = xT chunk -> yT in PSUM
  - column-sum  = free-dim reduce of xT
  - transmit+bias is a per-partition column in yT layout.

The x / w1 / w2 / y streams are bf16 (host casts both ways; rel err
~3e-3, well inside the 2e-2 gate) which HALVES the HBM traffic per
core.  All accumulation stays fp32 (PSUM / reduce accumulators).

Schedule: the collective latency (ncfw wake + cross-core launch-skew
barrier + AllGather, 40-120 us) dominates, so EVERYTHING that does not
depend on the global pooled vector runs before/during the wait:
  - a warm-up AllGather issued before the TileContext absorbs the ncfw
    wake + barrier from t~7us;
  - phase 1 streams x into SBUF (DMA-bound ~20 us) with column-sum
    slices on DVE/ACT; the 512 B local sum AllGathers at ~30 us;
  - the full matmul x@w1 runs during the wait, PSUM draining via pure
    copies into a resident bf16 staging buffer y0 (software-pipelined
    512-col matmuls / copies over one 8-bank PSUM tile).
After the AllGather lands, the gathered [8,128] block loads as one
clean 8-partition DMA, the cross-core sum is a single K=8 matmul with
a ones vector, and the only per-element work left is y0 + t -> out:
bf16 SBUF->SBUF tensor_scalar (DVE 2x/4x packing eligible) alternating
with ACT, feeding ramped output DMA chunks.  The post-collective tail
is then output-DMA-bound (~22 us) instead of Tensor-bound.
"""

import numpy as np
import ml_dtypes

import concourse.bass as bass
import concourse.tile as tile
from concourse import bacc, mybir
from concourse.bass_utils import run_bass_kernel_spmd

N_CORES = 8
D = 128                 # d_in == d_out
N_ROWS = 200000         # full set size
R = 25088               # padded rows per core: 8 * 25088 = 200704 >= 200000
IN_CHUNK = 8192         # columns per input DMA chunk (2 MiB bf16)
RED_CHUNK = 2048        # columns per column-sum reduce slice
MM_N = 512              # matmul free dim = copy-drain width (one PSUM bank)
PS_COLS = 4096          # one resident PSUM tile, all 8 banks
NPIPE = PS_COLS // MM_N  # matmuls run this many 512-col slices ahead of copies
OUT_CHUNKS = [2048, 4096, 8192, 8192, 2560]   # ramped output DMA chunks
assert sum(OUT_CHUNKS) == R

F32 = mybir.dt.float32
BF16 = mybir.dt.bfloat16
NP_BF16 = ml_dtypes.bfloat16


def _split(r, step, base=0):
    out = []
    c0 = 0
    while c0 < r:
        cw = min(step, r - c0)
        out.append((base + c0, cw))
        c0 += cw
    return out


def build_nc(r: int, n_total: int):
    """Build the SPMD Bass program for one core holding r rows."""
    in_chunks = _split(r, IN_CHUNK)

    nc = bacc.Bacc(
        "TRN2",
        target_bir_lowering=False,
        debug=False,
        num_devices=N_CORES,
    )

    xt = nc.declare_dram_parameter("xt", [D, r], BF16, isOutput=False)
    w1 = nc.declare_dram_parameter("w1", [D, D], BF16, isOutput=False)
    w2 = nc.declare_dram_parameter("w2", [D, D], BF16, isOutput=False)
    bias_c = nc.declare_dram_parameter("bias_c", [D, 1], F32, isOutput=False)
    ones8 = nc.declare_dram_parameter("ones8", [N_CORES, 1], F32, isOutput=False)
    out = nc.declare_dram_parameter("out", [D, r], BF16, isOutput=True)

    # Bounce buffers for the collectives (cannot touch I/O tensors).
    # AllGather output is rank-major: rank r's block at row r.
    cc_in = nc.dram_tensor("cc_in", [D, 1], F32)
    cc_out = nc.dram_tensor("cc_out", [N_CORES, D], F32, addr_space="Shared")
    ccw_in = nc.dram_tensor("ccw_in", [D, 1], F32)
    ccw_out = nc.dram_tensor("ccw_out", [N_CORES, D], F32, addr_space="Shared")

    # Warm-up collective, emitted BEFORE the TileContext so it is the very
    # first gpsimd instruction: wakes ncfw and enters the cross-core
    # barrier immediately at kernel start.  Content/result unused;
    # completion is guaranteed before the real AllGather because ncfw
    # serializes collectives in program order.
    warm_sem = nc.alloc_semaphore("warm_cc")
    nc.gpsimd.collective_compute(
        "AllGather",
        mybir.AluOpType.bypass,
        replica_groups=[list(range(N_CORES))],
        ins=[ccw_in.ap().opt()],
        outs=[ccw_out.ap().opt()],
    ).then_inc(warm_sem)

    with tile.TileContext(nc) as tc:
        with (
            tc.tile_pool(name="const", bufs=1) as const_pool,
            tc.tile_pool(name="xres", bufs=1) as xres_pool,
            tc.tile_pool(name="ystage", bufs=1) as y_pool,
            tc.tile_pool(name="obuf", bufs=3) as obuf_pool,
            tc.tile_pool(name="small", bufs=1) as small_pool,
            tc.tile_pool(name="mm", bufs=1, space=bass.MemorySpace.PSUM) as mm_pool,
        ):
            w1_sb = const_pool.tile([D, D], BF16)
            w2_sb = const_pool.tile([D, D], BF16)
            bias_sb = const_pool.tile([D, 1], F32)
            ones8_sb = const_pool.tile([N_CORES, 1], F32)
            nc.scalar.dma_start(w1_sb[:], w1[:, :])
            nc.scalar.dma_start(w2_sb[:], w2[:, :])
            nc.scalar.dma_start(bias_sb[:], bias_c[:, :])
            nc.scalar.dma_start(ones8_sb[:], ones8[:, :])

            x_sb = xres_pool.tile([D, r], BF16)
            y0 = y_pool.tile([D, r], BF16)

            # phase 1: stream ALL of xT into SBUF (alternating HWDGE rings)
            for c, (c0, cw) in enumerate(in_chunks):
                dma_eng = nc.sync if c % 2 == 0 else nc.scalar
                dma_eng.dma_start(x_sb[:, c0 : c0 + cw], xt[:, c0 : c0 + cw])

            # column-sum slices, emitted BEFORE the copy-drains so they
            # lead each engine's FIFO; the last slice + combine go on ACT
            # so DVE's copy stream starts as early as possible.
            red_slices = _split(r, RED_CHUNK)
            n_red = len(red_slices)
            cs_parts = small_pool.tile([D, n_red], F32)
            trash = small_pool.tile([D, RED_CHUNK], BF16)
            for s, (s0, sw) in enumerate(red_slices):
                if s % 2 == 0 and s != n_red - 1:
                    nc.vector.reduce_sum(
                        cs_parts[:, s : s + 1],
                        x_sb[:, s0 : s0 + sw],
                        axis=mybir.AxisListType.X,
                    )
                else:
                    nc.scalar.activation(
                        trash[:, :sw],
                        x_sb[:, s0 : s0 + sw],
                        mybir.ActivationFunctionType.Copy,
                        accum_out=cs_parts[:, s : s + 1],
                    )
            cs = small_pool.tile([D, 1], F32)
            nc.scalar.activation(
                trash[:, :n_red],
                cs_parts[:],
                mybir.ActivationFunctionType.Copy,
                accum_out=cs[:],
            )
            nc.sync.dma_start(cc_in[:, :], cs[:])

            nc.gpsimd.collective_compute(
                "AllGather",
                mybir.AluOpType.bypass,
                replica_groups=[list(range(N_CORES))],
                ins=[cc_in.ap().opt()],
                outs=[cc_out.ap().opt()],
            )

            # gathered [8, D] block loads as one clean 8-partition DMA
            g8 = small_pool.tile([N_CORES, D], F32)
            nc.sync.dma_start(g8[:, :], cc_out[:, :])

            # staged matmul: x@w1 -> PSUM -> pure-copy drain into y0, all
            # independent of the collective.  Software-pipelined emission:
            # copy(k) after mm(k); mm(k+NPIPE) — which overwrites group
            # k's PSUM bank — after copy(k).
            ps = mm_pool.tile([D, PS_COLS], F32, tag="ps")
            slices = _split(r, MM_N)

            def emit_mm(idx):
                s0, sw = slices[idx]
                po = s0 % PS_COLS
                nc.tensor.matmul(ps[:, po : po + sw], w1_sb[:], x_sb[:, s0 : s0 + sw])

            for k in range(min(NPIPE, len(slices))):
                emit_mm(k)
            for k, (s0, sw) in enumerate(slices):
                po = s0 % PS_COLS
                if k % 2 == 0:
                    nc.vector.tensor_copy(out=y0[:, s0 : s0 + sw], in_=ps[:, po : po + sw])
                else:
                    nc.scalar.activation(
                        y0[:, s0 : s0 + sw],
                        ps[:, po : po + sw],
                        mybir.ActivationFunctionType.Copy,
                    )
                if k + NPIPE < len(slices):
                    emit_mm(k + NPIPE)

            # cross-core sum: pool = ones8-weighted sum of the 8 gathered
            # rows (single K=8 matmul); then t = (w2.T @ pool/n) + bias,
            # produced in both bf16 (DVE adds) and fp32 (ACT bias path).
            nc.tensor.matmul(ps[:, 0:1], g8[:], ones8_sb[:])
            gcs = small_pool.tile([D, 1], BF16)
            nc.vector.tensor_scalar(
                out=gcs[:],
                in0=ps[:, 0:1],
                scalar1=1.0 / float(n_total),
                scalar2=None,
                op0=mybir.AluOpType.mult,
            )
            nc.tensor.matmul(ps[:, 1:2], w2_sb[:], gcs[:])
            t_f32 = small_pool.tile([D, 1], F32)
            nc.vector.tensor_scalar(
                out=t_f32[:],
                in0=ps[:, 1:2],
                scalar1=bias_sb[:],
                scalar2=None,
                op0=mybir.AluOpType.add,
            )

            # output: ob = y0 + t (bf16 SBUF->SBUF, alternating DVE/ACT),
            # then ramped DMA chunks on alternating rings.
            c0 = 0
            for c, cw in enumerate(OUT_CHUNKS):
                ob = obuf_pool.tile([D, max(OUT_CHUNKS)], BF16)
                if c % 2 == 0:
                    nc.vector.tensor_scalar(
                        out=ob[:, :cw],
                        in0=y0[:, c0 : c0 + cw],
                        scalar1=t_f32[:],
                        scalar2=None,
                        op0=mybir.AluOpType.add,
                    )
                else:
                    nc.scalar.activation(
                        ob[:, :cw],
                        y0[:, c0 : c0 + cw],
                        mybir.ActivationFunctionType.Identity,
                        bias=t_f32[:],
                        scale=1.0,
                    )
                (nc.sync if c % 2 == 0 else nc.scalar).dma_start(
                    out[:, c0 : c0 + cw], ob[:, :cw]
                )
                c0 += cw

    nc.compile()
    return nc


_nc_cache: dict = {}


def _get_nc(r: int, n_total: int):
    key = (r, n_total)
    if key not in _nc_cache:
        _nc_cache[key] = build_nc(r, n_total)
    return _nc_cache[key]


LAST_RESULTS = None


def _execute(x, w1, w2, bias, r, trace=False, tmpdir=None, trace_cores=None):
    global LAST_RESULTS
    x = np.ascontiguousarray(np.asarray(x, dtype=np.float32))
    w1 = np.asarray(w1, dtype=np.float32)
    w2 = np.asarray(w2, dtype=np.float32)
    bias = np.asarray(bias, dtype=np.float32)
    n, d = x.shape
    assert d == D and r * N_CORES >= n

    xp = np.zeros((N_CORES * r, d), dtype=np.float32)
    xp[:n] = x
    # (8, r, d) -> (8, d, r) pre-transposed bf16 shards
    xts = np.ascontiguousarray(
        xp.reshape(N_CORES, r, d).transpose(0, 2, 1).astype(NP_BF16)
    )
    w1_b = np.ascontiguousarray(w1.astype(NP_BF16))
    w2_b = np.ascontiguousarray(w2.astype(NP_BF16))
    bias_col = np.ascontiguousarray(bias.reshape(1, d).T)
    ones8_col = np.ones((N_CORES, 1), dtype=np.float32)

    in_maps = [
        {"xt": xts[i], "w1": w1_b, "w2": w2_b, "bias_c": bias_col, "ones8": ones8_col}
        for i in range(N_CORES)
    ]

    nc = _get_nc(r, n)
    kwargs = {}
    if trace:
        kwargs.update(trace=True, tmpdir=tmpdir)
        if trace_cores is not None:
            kwargs.update(trace_cores=trace_cores)
    res = run_bass_kernel_spmd(nc, in_maps, core_ids=list(range(N_CORES)), **kwargs)
    LAST_RESULTS = res

    yts = [res.results[i]["out"] for i in range(N_CORES)]  # each (D, r) bf16
    y = np.concatenate([yt.T.astype(np.float32) for yt in yts], axis=0)[:n]
    return np.ascontiguousarray(y)


def kernel(x, w1, w2, bias):
    return _execute(x, w1, w2, bias, R)
